# revision 1
# baseline (speedup 1.0000x reference)
"""nn_Block_15857019256918: windowed-attention transformer block on 8 trn2 cores.

Sharding: data-parallel over the B*25=100 attention windows (padded to 104 so
each of the 8 cores owns 13 windows). Every real token (b,h,w) belongs to
exactly one 14x14 window, so the residual + MLP for those tokens stays on the
same core — no cross-core communication at all. Weights are replicated.

Host does layout-only work (pad, window partition, static rel-pos gather,
unpartition); all FLOPs run on the 8 NeuronCores via one pmapped program.
"""

import numpy as np
import jax
import jax.numpy as jnp

DIM = 768
NH = 12
HD = DIM // NH
WS = 14
EPS = 1e-5
B, H, W = 4, 64, 64
NWIN_SIDE = 5           # ceil(64/14)
NWIN = B * NWIN_SIDE * NWIN_SIDE   # 100
NCORES = 8
NWIN_PAD = 104          # 8 * 13
N = WS * WS             # 196


def _ln(x, w, b):
    m = jnp.mean(x, -1, keepdims=True)
    v = jnp.var(x, -1, keepdims=True)
    return (x - m) * jax.lax.rsqrt(v + EPS) * w + b


def _core_fn(xw, mask, ln1_w, ln1_b, qkv_w, qkv_b, proj_w, proj_b,
             Rh, Rw, ln2_w, ln2_b, fc1_w, fc1_b, fc2_w, fc2_b):
    # xw: [nw, N, DIM] raw tokens (zero in pad region); mask: [nw, N, 1]
    # Heavy GEMMs run with bf16 operands + f32 accumulation (2x TensorE rate);
    # LN, softmax, gelu, residuals stay f32.
    bf = jnp.bfloat16
    f32 = jnp.float32
    nw = xw.shape[0]
    xn = _ln(xw, ln1_w, ln1_b) * mask          # pad rows forced to 0, as in ref

    qkv = jnp.matmul(xn.astype(bf), qkv_w.astype(bf),
                     preferred_element_type=f32) + qkv_b
    qkv = qkv.reshape(nw, N, 3, NH, HD).transpose(2, 0, 3, 1, 4)
    q, k, v = qkv[0], qkv[1], qkv[2]            # [nw, NH, N, HD] f32
    scale = HD ** -0.5
    attn = jnp.einsum("bhnd,bhmd->bhnm", (q * scale).astype(bf), k.astype(bf),
                      preferred_element_type=f32)

    rq = q.reshape(nw, NH, WS, WS, HD)
    rel_h = jnp.einsum("bnhwc,hkc->bnhwk", rq, Rh)
    rel_w = jnp.einsum("bnhwc,wkc->bnhwk", rq, Rw)
    attn = (attn.reshape(nw, NH, WS, WS, WS, WS)
            + rel_h[..., :, None] + rel_w[..., None, :]).reshape(nw, NH, N, N)

    attn = jax.nn.softmax(attn, axis=-1)
    out = jnp.einsum("bhnm,bhmd->bhnd", attn.astype(bf), v.astype(bf),
                     preferred_element_type=f32)
    out = out.transpose(0, 2, 1, 3).reshape(nw, N, DIM)
    out = jnp.matmul(out.astype(bf), proj_w.astype(bf),
                     preferred_element_type=f32) + proj_b

    tok = xw + out                              # residual (pad rows are garbage, dropped later)

    h = _ln(tok, ln2_w, ln2_b)
    h = jax.nn.gelu(jnp.matmul(h.astype(bf), fc1_w.astype(bf),
                               preferred_element_type=f32) + fc1_b,
                    approximate=False)
    return tok + (jnp.matmul(h.astype(bf), fc2_w.astype(bf),
                             preferred_element_type=f32) + fc2_b)


_pmapped = None


def _get_pmapped():
    global _pmapped
    if _pmapped is None:
        _pmapped = jax.pmap(
            _core_fn,
            in_axes=(0, 0) + (None,) * 14,
            devices=jax.devices()[:NCORES],
        )
    return _pmapped


def kernel(x, ln1_w, ln1_b, qkv_w, qkv_b, proj_w, proj_b,
           rel_pos_h, rel_pos_w, ln2_w, ln2_b, fc1_w, fc1_b, fc2_w, fc2_b):
    x = np.asarray(x, np.float32)

    # ---- host: window partition (layout only) ----
    xp = np.zeros((B, 70, 70, DIM), np.float32)
    xp[:, :H, :W, :] = x
    xw = xp.reshape(B, NWIN_SIDE, WS, NWIN_SIDE, WS, DIM).transpose(0, 1, 3, 2, 4, 5)
    xw = xw.reshape(NWIN, N, DIM)
    xw_pad = np.zeros((NWIN_PAD, N, DIM), np.float32)
    xw_pad[:NWIN] = xw
    xw_sh = xw_pad.reshape(NCORES, NWIN_PAD // NCORES, N, DIM)

    # per-window-position validity mask (1=real token, 0=pad)
    hreal = np.minimum(WS, H - WS * np.arange(NWIN_SIDE))        # [5]
    rowm = (np.arange(WS)[None, :] < hreal[:, None]).astype(np.float32)  # [5,14]
    m2 = np.einsum("ri,cj->rcij", rowm, rowm).reshape(NWIN_SIDE, NWIN_SIDE, N, 1)
    mask = np.broadcast_to(m2[None], (B, NWIN_SIDE, NWIN_SIDE, N, 1)).reshape(NWIN, N, 1)
    mask_pad = np.zeros((NWIN_PAD, N, 1), np.float32)
    mask_pad[:NWIN] = mask
    mask_sh = mask_pad.reshape(NCORES, NWIN_PAD // NCORES, N, 1)

    # static relative-position gather on host (indices depend only on shapes)
    idx = np.arange(WS)[:, None] - np.arange(WS)[None, :] + (WS - 1)
    Rh = np.asarray(rel_pos_h, np.float32)[idx]   # [WS, WS, HD]
    Rw = np.asarray(rel_pos_w, np.float32)[idx]

    out_sh = _get_pmapped()(
        xw_sh, mask_sh,
        jnp.asarray(ln1_w), jnp.asarray(ln1_b),
        jnp.asarray(qkv_w), jnp.asarray(qkv_b),
        jnp.asarray(proj_w), jnp.asarray(proj_b),
        jnp.asarray(Rh), jnp.asarray(Rw),
        jnp.asarray(ln2_w), jnp.asarray(ln2_b),
        jnp.asarray(fc1_w), jnp.asarray(fc1_b),
        jnp.asarray(fc2_w), jnp.asarray(fc2_b),
    )
    out = np.asarray(out_sh, np.float32).reshape(NWIN_PAD, N, DIM)[:NWIN]

    # ---- host: window unpartition + crop ----
    out = out.reshape(B, NWIN_SIDE, NWIN_SIDE, WS, WS, DIM).transpose(0, 1, 3, 2, 4, 5)
    out = out.reshape(B, 70, 70, DIM)[:, :H, :W, :]
    return np.ascontiguousarray(out, np.float32)



# revision 16
# speedup vs baseline: 39.4055x; 39.4055x over previous
"""nn_Block_15857019256918: windowed-attention transformer block on 8 trn2 cores.

Data-parallel over the B*25=100 attention windows (padded to 104; 13 windows
x 196 tokens = 2548 tokens per core). Hand-written Bass/Tile kernel:

  P0  LN1 (token-major, affine folded into weights host-side) -> PE-transpose
      -> xnT bf16 [768, 2548]
  P1  qT = Wq'^T @ xnT   (Wq prescaled by softmax scale 1/8)
  P2  rel-pos tables UT[h] = [rel_hT; rel_wT] (28 x 2548 per head) via
      per-(head, side, query-group) matmuls against host-expanded Rh/Rw
  P3  per window: kT, v (token-major, ones-column augmented for colsums)
  P4  per (window, head): ST = kT^T q accumulated with rank-28 bias add
      (Vmat^T UT) in psum; P = exp(ST) (no max-sub; |scores| < 3);
      outT/colsum via augmented AV matmul; normalize with DMA-broadcast
      reciprocal of the colsum row
  P5  proj token-major + residual -> tok (DRAM bounce)
  P6  LN2 -> PE-transpose -> fc1+Gelu -> fc2 + residual -> out

All matmuls bf16 with f32 psum accumulation; LN/softmax statistics f32.
"""

import numpy as np
import ml_dtypes

import concourse.bass as bass
import concourse.tile as tile
from concourse import bacc, mybir
from concourse.bass_utils import run_bass_kernel_spmd
from concourse.masks import make_identity

F32 = mybir.dt.float32
BF16 = mybir.dt.bfloat16
AF = mybir.ActivationFunctionType
ALU = mybir.AluOpType

DIM, NH, HD, WS = 768, 12, 64, 14
B, H, W = 4, 64, 64
NS = 5
NWIN = B * NS * NS          # 100
NCORES = 8
WPC = 13                    # windows per core
N = WS * WS                 # 196
TPC = WPC * N               # 2548
EPS = 1e-5
NC6 = 6                     # DIM // 128
bfloat16 = ml_dtypes.bfloat16

CHUNKS = [(i * 128, min(128, TPC - i * 128)) for i in range((TPC + 127) // 128)]
NCHUNK = [(i * 512, min(512, TPC - i * 512)) for i in range((TPC + 511) // 512)]
MROWS = [(0, 128), (128, 68)]   # per-window token chunks


def build_program():
    nc = bacc.Bacc("TRN2", target_bir_lowering=False, debug=False)

    x_d = nc.dram_tensor("xc", [TPC, DIM], F32, kind="ExternalInput")
    out_d = nc.dram_tensor("out", [TPC, DIM], F32, kind="ExternalOutput")
    wq_d = nc.dram_tensor("wq", [DIM, DIM], BF16, kind="ExternalInput")
    wk_d = nc.dram_tensor("wk", [DIM, DIM], BF16, kind="ExternalInput")
    wv_d = nc.dram_tensor("wv", [DIM, DIM], BF16, kind="ExternalInput")
    wp_d = nc.dram_tensor("wp", [DIM, DIM], BF16, kind="ExternalInput")
    w1_d = nc.dram_tensor("w1", [DIM, 4 * DIM], BF16, kind="ExternalInput")
    w2_d = nc.dram_tensor("w2", [4 * DIM, DIM], BF16, kind="ExternalInput")
    rhw_d = nc.dram_tensor("rhw", [HD, WS * 2 * WS], BF16, kind="ExternalInput")
    vmat_d = nc.dram_tensor("vmat", [2 * WS, N], BF16, kind="ExternalInput")
    bq_d = nc.dram_tensor("bq", [128, NC6], F32, kind="ExternalInput")
    bk_d = nc.dram_tensor("bk", [128, NC6], F32, kind="ExternalInput")
    b1_d = nc.dram_tensor("b1", [128, 4 * NC6], F32, kind="ExternalInput")
    bv_d = nc.dram_tensor("bv", [1, DIM], BF16, kind="ExternalInput")
    bp_d = nc.dram_tensor("bp", [1, DIM], BF16, kind="ExternalInput")
    b2_d = nc.dram_tensor("b2", [1, DIM], BF16, kind="ExternalInput")
    tok_d = nc.dram_tensor("tok", [TPC, DIM], F32)

    with tile.TileContext(nc) as tc:
        _build(nc, tc, locals())
    nc.compile()
    return nc


def _build(nc, tc, d):
    import contextlib
    ctx = contextlib.ExitStack()
    x_d, out_d, tok_d = d["x_d"], d["out_d"], d["tok_d"]

    persist = ctx.enter_context(tc.tile_pool(name="persist", bufs=1))
    xpool = ctx.enter_context(tc.tile_pool(name="xpool", bufs=2))
    stat = ctx.enter_context(tc.tile_pool(name="stat", bufs=4))
    wpool = ctx.enter_context(tc.tile_pool(name="wpool", bufs=2))
    hpool = ctx.enter_context(tc.tile_pool(name="hpool", bufs=2))
    htpool = ctx.enter_context(tc.tile_pool(name="htpool", bufs=1))
    smal = ctx.enter_context(tc.tile_pool(name="smal", bufs=2))
    # PSUM: 4 tags x bufs=2 = 8 banks exactly
    psA = ctx.enter_context(tc.tile_pool(name="psA", bufs=2, space="PSUM"))
    psB = ctx.enter_context(tc.tile_pool(name="psB", bufs=2, space="PSUM"))

    def ps_tile(shape, tag, dtype=F32):
        pool = psA if tag in ("tp", "mm") else psB
        return pool.tile(shape, dtype, tag=tag, name=f"ps_{tag}")

    # ---------------- constants / weights ----------------
    ident = persist.tile([128, 128], BF16)
    make_identity(nc, ident)
    eps_t = persist.tile([128, 1], F32)
    nc.vector.memset(eps_t, EPS)
    ones1 = persist.tile([1, 128], BF16)
    nc.vector.memset(ones1, 1.0)

    def load_w(dram, n_k, n_cols, tagf):
        ts_ = []
        for i in range(n_k):
            t = persist.tile([128, n_cols], BF16, tag=tagf(i), name=f"w_{tagf(i)}")
            nc.sync.dma_start(out=t, in_=dram[i * 128:(i + 1) * 128, :])
            ts_.append(t)
        return ts_

    wq_s = load_w(d["wq_d"], NC6, DIM, lambda i: f"wg{i}")
    wk_s = load_w(d["wk_d"], NC6, DIM, lambda i: f"wg{i + 6}")
    wv_s = load_w(d["wv_d"], NC6, DIM, lambda i: f"wg{i + 12}")
    wp_s = load_w(d["wp_d"], NC6, DIM, lambda i: f"wg{i + 18}")
    # duplicated at both 64-partition offsets so lhsT/rhs bases can match
    rhw_s = persist.tile([128, WS * 2 * WS], BF16)
    nc.sync.dma_start(out=rhw_s[0:HD], in_=d["rhw_d"][:, :])
    nc.sync.dma_start(out=rhw_s[HD:128], in_=d["rhw_d"][:, :])
    # duplicated at the three 32-row offsets used by UT head slots
    vmat_s = persist.tile([92, N], BF16)
    for r in range(3):
        nc.sync.dma_start(out=vmat_s[r * 32:r * 32 + 28], in_=d["vmat_d"][:, :])
    bias_s = {}
    for nm, shp in (("bq", [128, NC6]), ("bk", [128, NC6]), ("b1", [128, 4 * NC6])):
        t = persist.tile(shp, F32, tag=nm, name=f"bias_{nm}")
        nc.sync.dma_start(out=t, in_=d[nm + "_d"][:, :])
        bias_s[nm] = t
    for nm in ("bv", "bp", "b2"):
        t = persist.tile([1, DIM], BF16, tag=nm, name=f"bias_{nm}")
        nc.sync.dma_start(out=t, in_=d[nm + "_d"][:, :])
        bias_s[nm] = t

    # ---------------- persistent activations ----------------
    xnT = [persist.tile([128, TPC], BF16, tag=f"xnT{i}", name=f"xnT{i}") for i in range(NC6)]
    qT = [persist.tile([128, TPC], BF16, tag=f"qT{i}", name=f"qT{i}") for i in range(NC6)]
    UT = [persist.tile([92, TPC], BF16, tag=f"UT{i}", name=f"UT{i}") for i in range(4)]

    def ln_normalize(xt, rows, out_bf):
        stats = stat.tile([128, 3, NC6], F32, tag="bnst")
        for sg in range(3):
            nc.vector.bn_stats(out=stats[:rows, sg, :],
                               in_=xt[:rows, sg * 256:(sg + 1) * 256])
        mv = stat.tile([128, 2], F32, tag="mv")
        nc.vector.bn_aggr(out=mv[:rows], in_=stats[:rows])
        sd = stat.tile([128, 1], F32, tag="sd")
        nc.scalar.activation(out=sd[:rows], in_=mv[:rows, 1:2], func=AF.Sqrt,
                             bias=eps_t[:rows])
        rstd = stat.tile([128, 1], F32, tag="rstd")
        nc.vector.reciprocal(out=rstd[:rows], in_=sd[:rows])
        nc.vector.tensor_scalar(out=out_bf[:rows], in0=xt[:rows],
                                scalar1=mv[:rows, 0:1], scalar2=rstd[:rows],
                                op0=ALU.subtract, op1=ALU.mult)

    def ln_transpose_phase(src_dram, dstT):
        for t0, rows in CHUNKS:
            xt = xpool.tile([128, DIM], F32, tag="xin")
            nc.sync.dma_start(out=xt[:rows], in_=src_dram[t0:t0 + rows, :])
            xn_bf = xpool.tile([128, DIM], BF16, tag="xnbf")
            ln_normalize(xt, rows, xn_bf)
            for c in range(NC6):
                pt = ps_tile([128, 128], "tp", BF16)
                nc.tensor.transpose(pt[:, :rows],
                                    xn_bf[:rows, c * 128:(c + 1) * 128],
                                    ident[:rows, :rows])
                nc.scalar.copy(out=dstT[c][:, t0:t0 + rows], in_=pt[:, :rows])

    # ================ P0 ================
    ln_transpose_phase(x_d, xnT)

    # ================ P1: qT ================
    for j in range(NC6):
        for n0, ncols in NCHUNK:
            ps = ps_tile([128, 512], "mm")
            for c in range(NC6):
                nc.tensor.matmul(ps[:, :ncols], wq_s[c][:, j * 128:(j + 1) * 128],
                                 xnT[c][:, n0:n0 + ncols],
                                 start=(c == 0), stop=(c == NC6 - 1))
            nc.scalar.activation(out=qT[j][:, n0:n0 + ncols], in_=ps[:, :ncols],
                                 func=AF.Identity, bias=bias_s["bq"][:, j:j + 1])

    # ================ P2: rel tables ================
    for h in range(NH):
        ut = UT[h // 3]
        r0 = (h % 3) * 32
        qsl = qT[h // 2][(h % 2) * 64:(h % 2) * 64 + 64, :].rearrange(
            "p (w a b) -> p w a b", a=WS, b=WS)
        usl = ut[r0:r0 + 28, :].rearrange("p (w a b) -> p w a b", a=WS, b=WS)
        relw_tmp = hpool.tile([14, TPC], BF16, tag="relw", name="relw_tmp", bufs=1)
        wsl = relw_tmp.rearrange("p (w a b) -> p w a b", a=WS, b=WS)
        qrow = (h % 2) * 64
        for side in range(2):
            for g in range(WS):
                src = qsl[:, :, g, :] if side == 0 else qsl[:, :, :, g]
                ps = ps_tile([WS, WPC * WS], "st")
                nc.tensor.matmul(
                    ps[:, :],
                    rhw_s[qrow:qrow + HD, g * 28 + side * 14:g * 28 + side * 14 + 14],
                    src, start=True, stop=True)
                if side == 0:
                    nc.scalar.copy(out=usl[0:14, :, g, :], in_=ps[:, :])
                else:
                    # dst base partition r0+14 is illegal for ACT/DVE: stage in
                    # a base-0 temp, then DMA (partition-unconstrained) to UT
                    nc.scalar.copy(out=wsl[:, :, :, g], in_=ps[:, :])
        nc.sync.dma_start(out=ut[r0 + 14:r0 + 28, :], in_=relw_tmp[:, :])

    # ================ P3-5: per window ================
    for w in range(WPC):
        c0 = w * N
        kt = [wpool.tile([128, N], BF16, tag=f"kt{j}", name=f"kt{j}") for j in range(NC6)]
        for j in range(NC6):
            ps = ps_tile([128, 512], "mm")
            for c in range(NC6):
                nc.tensor.matmul(ps[:, :N], wk_s[c][:, j * 128:(j + 1) * 128],
                                 xnT[c][:, c0:c0 + N],
                                 start=(c == 0), stop=(c == NC6 - 1))
            nc.scalar.activation(out=kt[j][:, :], in_=ps[:, :N],
                                 func=AF.Identity, bias=bias_s["bk"][:, j:j + 1])

        va = [wpool.tile([128, NH * 65], BF16, tag=f"va{m}", name=f"va{m}") for m in range(2)]
        for m, (m0, mrows) in enumerate(MROWS):
            for f in range(2):
                ps = ps_tile([128, 512], "mm")
                for c in range(NC6):
                    nc.tensor.matmul(ps[:mrows, :384],
                                     xnT[c][:, c0 + m0:c0 + m0 + mrows],
                                     wv_s[c][:, f * 384:(f + 1) * 384],
                                     start=(c == 0), stop=(c == NC6 - 1))
                nc.tensor.matmul(ps[:mrows, :384], ones1[:, :mrows],
                                 bias_s["bv"][:, f * 384:(f + 1) * 384],
                                 start=False, stop=True, skip_group_check=True)
                dst = va[m].rearrange("p (h e) -> p h e", e=65)[
                    :mrows, f * 6:(f + 1) * 6, 0:64]
                nc.scalar.copy(out=dst, in_=ps[:mrows, :384])
            ones_cols = va[m].rearrange("p (h e) -> p h e", e=65)[:mrows, :, 64:65]
            nc.gpsimd.memset(ones_cols, 1.0)

        outT = [wpool.tile([128, N], BF16, tag=f"oT{j}", name=f"oT{j}") for j in range(NC6)]
        for h in range(NH):
            qtile, ktile = qT[h // 2], kt[h // 2]
            qrow = (h % 2) * 64
            ut, r0 = UT[h // 3], (h % 3) * 32
            pchunks = []
            for m0, mrows in MROWS:
                ps = ps_tile([128, N], "st")
                nc.tensor.matmul(ps[:mrows, :],
                                 ktile[qrow:qrow + 64, m0:m0 + mrows],
                                 qtile[qrow:qrow + 64, c0:c0 + N],
                                 start=True, stop=False, skip_group_check=True)
                nc.tensor.matmul(ps[:mrows, :], vmat_s[r0:r0 + 28, m0:m0 + mrows],
                                 ut[r0:r0 + 28, c0:c0 + N],
                                 start=False, stop=True, skip_group_check=True)
                pt = hpool.tile([128, N], BF16, tag="pexp")
                nc.scalar.activation(out=pt[:mrows], in_=ps[:mrows], func=AF.Exp)
                pchunks.append((pt, mrows))
            av = ps_tile([65, N], "av")
            for i, (pt, mrows) in enumerate(pchunks):
                nc.tensor.matmul(av[:, :], va[i][:mrows, h * 65:h * 65 + 65],
                                 pt[:mrows, :], start=(i == 0), stop=(i == 1),
                                 skip_group_check=True)
            rs = smal.tile([1, N], F32, tag="rs")
            nc.vector.reciprocal(out=rs, in_=av[64:65, :])
            rsb = smal.tile([64, N], F32, tag="rsb")
            nc.gpsimd.partition_broadcast(rsb, rs)
            nc.vector.tensor_mul(
                out=outT[h // 2][(h % 2) * 64:(h % 2) * 64 + 64, :],
                in0=av[0:64, :], in1=rsb)

        for m0, mrows in MROWS:
            tok_sb = hpool.tile([128, DIM], F32, tag="toksb")
            xres = hpool.tile([128, DIM], F32, tag="xres")
            nc.sync.dma_start(out=xres[:mrows], in_=x_d[c0 + m0:c0 + m0 + mrows, :])
            for f in range(2):
                ps = ps_tile([128, 512], "mm")
                for c in range(NC6):
                    nc.tensor.matmul(ps[:mrows, :384],
                                     outT[c][:, m0:m0 + mrows],
                                     wp_s[c][:, f * 384:(f + 1) * 384],
                                     start=(c == 0), stop=(c == NC6 - 1))
                nc.tensor.matmul(ps[:mrows, :384], ones1[:, :mrows],
                                 bias_s["bp"][:, f * 384:(f + 1) * 384],
                                 start=False, stop=True, skip_group_check=True)
                nc.vector.tensor_add(out=tok_sb[:mrows, f * 384:(f + 1) * 384],
                                     in0=ps[:mrows, :384],
                                     in1=xres[:mrows, f * 384:(f + 1) * 384])
            nc.sync.dma_start(out=tok_d[c0 + m0:c0 + m0 + mrows, :],
                              in_=tok_sb[:mrows])

    # ================ P6: LN2 + MLP ================
    w1_s = load_w(d["w1_d"], NC6, 4 * DIM, lambda i: f"xnT{i}")
    w2_s = load_w(d["w2_d"], 24, DIM, lambda i: f"wg{i}")
    xn2T = [persist.tile([128, TPC], BF16, tag=f"qT{i}", name=f"xn2T{i}") for i in range(NC6)]

    ln_transpose_phase(tok_d, xn2T)

    for n0, ncols in NCHUNK:
        ht = [htpool.tile([128, 512], BF16, tag=f"ht{f}", name=f"ht{f}") for f in range(24)]
        for f in range(24):
            ps = ps_tile([128, 512], "mm")
            for c in range(NC6):
                nc.tensor.matmul(ps[:, :ncols], w1_s[c][:, f * 128:(f + 1) * 128],
                                 xn2T[c][:, n0:n0 + ncols],
                                 start=(c == 0), stop=(c == NC6 - 1))
            nc.scalar.activation(out=ht[f][:, :ncols], in_=ps[:, :ncols],
                                 func=AF.Gelu, bias=bias_s["b1"][:, f:f + 1])
        nsub = [(s * 128, min(128, ncols - s * 128))
                for s in range((ncols + 127) // 128)]
        for s0, srows in nsub:
            tok_sb = hpool.tile([128, DIM], F32, tag="toksb")
            nc.sync.dma_start(out=tok_sb[:srows],
                              in_=tok_d[n0 + s0:n0 + s0 + srows, :])
            y_sb = hpool.tile([128, DIM], F32, tag="ysb")
            for half in range(2):
                ps = ps_tile([128, 512], "av")
                for f in range(24):
                    nc.tensor.matmul(ps[:srows, :384],
                                     ht[f][:, s0:s0 + srows],
                                     w2_s[f][:, half * 384:(half + 1) * 384],
                                     start=(f == 0), stop=(f == 23))
                nc.tensor.matmul(ps[:srows, :384], ones1[:, :srows],
                                 bias_s["b2"][:, half * 384:(half + 1) * 384],
                                 start=False, stop=True, skip_group_check=True)
                nc.vector.tensor_add(
                    out=y_sb[:srows, half * 384:(half + 1) * 384],
                    in0=ps[:srows, :384],
                    in1=tok_sb[:srows, half * 384:(half + 1) * 384])
            nc.sync.dma_start(out=out_d[n0 + s0:n0 + s0 + srows, :],
                              in_=y_sb[:srows])
    ctx.close()


# ================= host side =================

_nc = None


def _get_nc():
    global _nc
    if _nc is None:
        _nc = build_program()
    return _nc


def _prep_host(inputs):
    f32 = np.float32
    qkv_w = np.asarray(inputs["qkv_w"], f32)
    qkv_b = np.asarray(inputs["qkv_b"], f32)
    ln1_w = np.asarray(inputs["ln1_w"], f32)
    ln1_b = np.asarray(inputs["ln1_b"], f32)
    ln2_w = np.asarray(inputs["ln2_w"], f32)
    ln2_b = np.asarray(inputs["ln2_b"], f32)
    Wq = (ln1_w[:, None] * qkv_w[:, :DIM]) / 8.0
    Wk = ln1_w[:, None] * qkv_w[:, DIM:2 * DIM]
    Wv = ln1_w[:, None] * qkv_w[:, 2 * DIM:]
    bq = (qkv_b[:DIM] + ln1_b @ qkv_w[:, :DIM]) / 8.0
    bk = qkv_b[DIM:2 * DIM] + ln1_b @ qkv_w[:, DIM:2 * DIM]
    bv = qkv_b[2 * DIM:] + ln1_b @ qkv_w[:, 2 * DIM:]
    fc1_w = np.asarray(inputs["fc1_w"], f32)
    W1 = ln2_w[:, None] * fc1_w
    b1 = np.asarray(inputs["fc1_b"], f32) + ln2_b @ fc1_w

    idx = np.arange(WS)[:, None] - np.arange(WS)[None, :] + (WS - 1)
    Rh = np.asarray(inputs["rel_pos_h"], f32)[idx]
    Rw = np.asarray(inputs["rel_pos_w"], f32)[idx]
    rhw = np.zeros((HD, WS, 2 * WS), f32)          # [c, g, side*14+k]
    rhw[:, :, :WS] = 8.0 * Rh.transpose(2, 0, 1)
    rhw[:, :, WS:] = 8.0 * Rw.transpose(2, 0, 1)
    rhw = rhw.reshape(HD, WS * 2 * WS)
    m = np.arange(N)
    vmat = np.zeros((2 * WS, N), f32)
    for j in range(WS):
        vmat[j, (m // WS) == j] = 1.0
        vmat[WS + j, (m % WS) == j] = 1.0

    bf = bfloat16
    return {
        "wq": Wq.astype(bf), "wk": Wk.astype(bf), "wv": Wv.astype(bf),
        "wp": np.asarray(inputs["proj_w"], f32).astype(bf),
        "w1": W1.astype(bf),
        "w2": np.asarray(inputs["fc2_w"], f32).astype(bf),
        "rhw": rhw.astype(bf), "vmat": vmat.astype(bf),
        "bq": np.ascontiguousarray(bq.reshape(NC6, 128).T),
        "bk": np.ascontiguousarray(bk.reshape(NC6, 128).T),
        "b1": np.ascontiguousarray(b1.reshape(4 * NC6, 128).T),
        "bv": bv.reshape(1, DIM).astype(bf),
        "bp": np.asarray(inputs["proj_b"], f32).reshape(1, DIM).astype(bf),
        "b2": np.asarray(inputs["fc2_b"], f32).reshape(1, DIM).astype(bf),
    }


def _window_partition(x):
    xp = np.zeros((B, 70, 70, DIM), np.float32)
    xp[:, :H, :W, :] = x
    xw = xp.reshape(B, NS, WS, NS, WS, DIM).transpose(0, 1, 3, 2, 4, 5)
    xw = xw.reshape(NWIN, N, DIM)
    full = np.zeros((NCORES * WPC, N, DIM), np.float32)
    full[:NWIN] = xw
    return full


def _window_unpartition(y):
    yw = y[:NWIN].reshape(B, NS, NS, WS, WS, DIM).transpose(0, 1, 3, 2, 4, 5)
    return np.ascontiguousarray(yw.reshape(B, 70, 70, DIM)[:, :H, :W, :])


def make_in_maps(inputs):
    x = np.asarray(inputs["x"], np.float32)
    xw = _window_partition(x)
    wts = _prep_host(inputs)
    in_maps = []
    for c in range(NCORES):
        m = dict(wts)
        m["xc"] = np.ascontiguousarray(
            xw[c * WPC:(c + 1) * WPC].reshape(TPC, DIM))
        in_maps.append(m)
    return in_maps


def run_on_hw(inputs, trace=False, **kw):
    nc = _get_nc()
    in_maps = make_in_maps(inputs)
    res = run_bass_kernel_spmd(nc, in_maps, core_ids=list(range(NCORES)),
                               trace=trace, **kw)
    out = np.zeros((NCORES * WPC, N, DIM), np.float32)
    for c in range(NCORES):
        out[c * WPC:(c + 1) * WPC] = res.results[c]["out"].reshape(WPC, N, DIM)
    return _window_unpartition(out), res


def kernel(x, ln1_w, ln1_b, qkv_w, qkv_b, proj_w, proj_b,
           rel_pos_h, rel_pos_w, ln2_w, ln2_b, fc1_w, fc1_b, fc2_w, fc2_b):
    inputs = dict(x=x, ln1_w=ln1_w, ln1_b=ln1_b, qkv_w=qkv_w, qkv_b=qkv_b,
                  proj_w=proj_w, proj_b=proj_b, rel_pos_h=rel_pos_h,
                  rel_pos_w=rel_pos_w, ln2_w=ln2_w, ln2_b=ln2_b,
                  fc1_w=fc1_w, fc1_b=fc1_b, fc2_w=fc2_w, fc2_b=fc2_b)
    out, _ = run_on_hw(inputs, trace=False)
    return out


# revision 18
# speedup vs baseline: 39.6300x; 1.0057x over previous
"""nn_Block_15857019256918: windowed-attention transformer block on 8 trn2 cores.

Data-parallel over the B*25=100 attention windows (padded to 104; 13 windows
x 196 tokens = 2548 tokens per core). Hand-written Bass/Tile kernel:

  P0  LN1 (token-major, affine folded into weights host-side) -> PE-transpose
      -> xnT bf16 [768, 2548]
  P1  qT = Wq'^T @ xnT   (Wq prescaled by softmax scale 1/8)
  P2  rel-pos tables UT[h] = [rel_hT; rel_wT] (28 x 2548 per head) via
      per-(head, side, query-group) matmuls against host-expanded Rh/Rw
  P3  per window: kT, v (token-major, ones-column augmented for colsums)
  P4  per (window, head): ST = kT^T q accumulated with rank-28 bias add
      (Vmat^T UT) in psum; P = exp(ST) (no max-sub; |scores| < 3);
      outT/colsum via augmented AV matmul; normalize with DMA-broadcast
      reciprocal of the colsum row
  P5  proj token-major + residual -> tok (DRAM bounce)
  P6  LN2 -> PE-transpose -> fc1+Gelu -> fc2 + residual -> out

All matmuls bf16 with f32 psum accumulation; LN/softmax statistics f32.
"""

import numpy as np
import ml_dtypes

import concourse.bass as bass
import concourse.tile as tile
from concourse import bacc, mybir
from concourse.bass_utils import run_bass_kernel_spmd
from concourse.masks import make_identity

F32 = mybir.dt.float32
BF16 = mybir.dt.bfloat16
AF = mybir.ActivationFunctionType
ALU = mybir.AluOpType

DIM, NH, HD, WS = 768, 12, 64, 14
B, H, W = 4, 64, 64
NS = 5
NWIN = B * NS * NS          # 100
NCORES = 8
WPC = 13                    # windows per core
N = WS * WS                 # 196
TPC = WPC * N               # 2548
EPS = 1e-5
NC6 = 6                     # DIM // 128
bfloat16 = ml_dtypes.bfloat16

CHUNKS = [(i * 128, min(128, TPC - i * 128)) for i in range((TPC + 127) // 128)]
NCHUNK = [(i * 512, min(512, TPC - i * 512)) for i in range((TPC + 511) // 512)]
MROWS = [(0, 128), (128, 68)]   # per-window token chunks


def build_program():
    nc = bacc.Bacc("TRN2", target_bir_lowering=False, debug=False)

    x_d = nc.dram_tensor("xc", [TPC, DIM], F32, kind="ExternalInput")
    out_d = nc.dram_tensor("out", [TPC, DIM], F32, kind="ExternalOutput")
    wq_d = nc.dram_tensor("wq", [DIM, DIM], BF16, kind="ExternalInput")
    wk_d = nc.dram_tensor("wk", [DIM, DIM], BF16, kind="ExternalInput")
    wv_d = nc.dram_tensor("wv", [DIM, DIM], BF16, kind="ExternalInput")
    wp_d = nc.dram_tensor("wp", [DIM, DIM], BF16, kind="ExternalInput")
    w1_d = nc.dram_tensor("w1", [DIM, 4 * DIM], BF16, kind="ExternalInput")
    w2_d = nc.dram_tensor("w2", [4 * DIM, DIM], BF16, kind="ExternalInput")
    rhw_d = nc.dram_tensor("rhw", [HD, WS * 2 * WS], BF16, kind="ExternalInput")
    vmat_d = nc.dram_tensor("vmat", [2 * WS, N], BF16, kind="ExternalInput")
    bq_d = nc.dram_tensor("bq", [128, NC6], F32, kind="ExternalInput")
    bk_d = nc.dram_tensor("bk", [128, NC6], F32, kind="ExternalInput")
    b1_d = nc.dram_tensor("b1", [128, 4 * NC6], F32, kind="ExternalInput")
    bv_d = nc.dram_tensor("bv", [1, DIM], BF16, kind="ExternalInput")
    bp_d = nc.dram_tensor("bp", [1, DIM], BF16, kind="ExternalInput")
    b2_d = nc.dram_tensor("b2", [1, DIM], BF16, kind="ExternalInput")
    tok_d = nc.dram_tensor("tok", [TPC, DIM], F32)

    with tile.TileContext(nc) as tc:
        _build(nc, tc, locals())
    nc.compile()
    return nc


def _build(nc, tc, d):
    import contextlib
    ctx = contextlib.ExitStack()
    x_d, out_d, tok_d = d["x_d"], d["out_d"], d["tok_d"]

    persist = ctx.enter_context(tc.tile_pool(name="persist", bufs=1))
    xpool = ctx.enter_context(tc.tile_pool(name="xpool", bufs=2))
    stat = ctx.enter_context(tc.tile_pool(name="stat", bufs=4))
    wpool = ctx.enter_context(tc.tile_pool(name="wpool", bufs=2))
    hpool = ctx.enter_context(tc.tile_pool(name="hpool", bufs=2))
    htpool = ctx.enter_context(tc.tile_pool(name="htpool", bufs=1))
    smal = ctx.enter_context(tc.tile_pool(name="smal", bufs=2))
    # PSUM: 4 tags x bufs=2 = 8 banks exactly
    psA = ctx.enter_context(tc.tile_pool(name="psA", bufs=2, space="PSUM"))
    psB = ctx.enter_context(tc.tile_pool(name="psB", bufs=2, space="PSUM"))

    def ps_tile(shape, tag, dtype=F32):
        pool = psA if tag in ("tp", "mm") else psB
        return pool.tile(shape, dtype, tag=tag, name=f"ps_{tag}")

    # ---------------- constants / weights ----------------
    ident = persist.tile([128, 128], BF16)
    make_identity(nc, ident)
    eps_t = persist.tile([128, 1], F32)
    nc.vector.memset(eps_t, EPS)
    ones1 = persist.tile([1, 128], BF16)
    nc.vector.memset(ones1, 1.0)

    def load_w(dram, n_k, n_cols, tagf):
        ts_ = []
        for i in range(n_k):
            t = persist.tile([128, n_cols], BF16, tag=tagf(i), name=f"w_{tagf(i)}")
            nc.sync.dma_start(out=t, in_=dram[i * 128:(i + 1) * 128, :])
            ts_.append(t)
        return ts_

    wq_s = load_w(d["wq_d"], NC6, DIM, lambda i: f"wg{i}")
    wk_s = load_w(d["wk_d"], NC6, DIM, lambda i: f"wg{i + 6}")
    wv_s = load_w(d["wv_d"], NC6, DIM, lambda i: f"wg{i + 12}")
    wp_s = load_w(d["wp_d"], NC6, DIM, lambda i: f"wg{i + 18}")
    # duplicated at both 64-partition offsets so lhsT/rhs bases can match
    rhw_s = persist.tile([128, WS * 2 * WS], BF16)
    nc.sync.dma_start(out=rhw_s[0:HD], in_=d["rhw_d"][:, :])
    nc.sync.dma_start(out=rhw_s[HD:128], in_=d["rhw_d"][:, :])
    # duplicated at the three 32-row offsets used by UT head slots
    vmat_s = persist.tile([92, N], BF16)
    for r in range(3):
        nc.sync.dma_start(out=vmat_s[r * 32:r * 32 + 28], in_=d["vmat_d"][:, :])
    bias_s = {}
    for nm, shp in (("bq", [128, NC6]), ("bk", [128, NC6]), ("b1", [128, 4 * NC6])):
        t = persist.tile(shp, F32, tag=nm, name=f"bias_{nm}")
        nc.sync.dma_start(out=t, in_=d[nm + "_d"][:, :])
        bias_s[nm] = t
    for nm in ("bv", "bp", "b2"):
        t = persist.tile([128, DIM], BF16, tag=nm, name=f"bias_{nm}")
        nc.sync.dma_start(out=t, in_=d[nm + "_d"][:, :].to_broadcast((128, DIM)))
        bias_s[nm] = t

    # ---------------- persistent activations ----------------
    xnT = [persist.tile([128, TPC], BF16, tag=f"xnT{i}", name=f"xnT{i}") for i in range(NC6)]
    qT = [persist.tile([128, TPC], BF16, tag=f"qT{i}", name=f"qT{i}") for i in range(NC6)]
    UT = [persist.tile([92, TPC], BF16, tag=f"UT{i}", name=f"UT{i}") for i in range(4)]

    def ln_normalize(xt, rows, out_bf):
        stats = stat.tile([128, 3, NC6], F32, tag="bnst")
        for sg in range(3):
            nc.vector.bn_stats(out=stats[:rows, sg, :],
                               in_=xt[:rows, sg * 256:(sg + 1) * 256])
        mv = stat.tile([128, 2], F32, tag="mv")
        nc.vector.bn_aggr(out=mv[:rows], in_=stats[:rows])
        sd = stat.tile([128, 1], F32, tag="sd")
        nc.scalar.activation(out=sd[:rows], in_=mv[:rows, 1:2], func=AF.Sqrt,
                             bias=eps_t[:rows])
        rstd = stat.tile([128, 1], F32, tag="rstd")
        nc.vector.reciprocal(out=rstd[:rows], in_=sd[:rows])
        nc.vector.tensor_scalar(out=out_bf[:rows], in0=xt[:rows],
                                scalar1=mv[:rows, 0:1], scalar2=rstd[:rows],
                                op0=ALU.subtract, op1=ALU.mult)

    def ln_transpose_phase(src_dram, dstT):
        for t0, rows in CHUNKS:
            xt = xpool.tile([128, DIM], F32, tag="xin")
            nc.sync.dma_start(out=xt[:rows], in_=src_dram[t0:t0 + rows, :])
            xn_bf = xpool.tile([128, DIM], BF16, tag="xnbf")
            ln_normalize(xt, rows, xn_bf)
            for c in range(NC6):
                pt = ps_tile([128, 128], "tp", BF16)
                nc.tensor.transpose(pt[:, :rows],
                                    xn_bf[:rows, c * 128:(c + 1) * 128],
                                    ident[:rows, :rows])
                nc.scalar.copy(out=dstT[c][:, t0:t0 + rows], in_=pt[:, :rows])

    # ================ P0 ================
    ln_transpose_phase(x_d, xnT)

    # ================ P1: qT ================
    for j in range(NC6):
        for n0, ncols in NCHUNK:
            ps = ps_tile([128, 512], "mm")
            for c in range(NC6):
                nc.tensor.matmul(ps[:, :ncols], wq_s[c][:, j * 128:(j + 1) * 128],
                                 xnT[c][:, n0:n0 + ncols],
                                 start=(c == 0), stop=(c == NC6 - 1))
            nc.scalar.activation(out=qT[j][:, n0:n0 + ncols], in_=ps[:, :ncols],
                                 func=AF.Identity, bias=bias_s["bq"][:, j:j + 1])

    # ================ P2: rel tables ================
    for h in range(NH):
        ut = UT[h // 3]
        r0 = (h % 3) * 32
        qsl = qT[h // 2][(h % 2) * 64:(h % 2) * 64 + 64, :].rearrange(
            "p (w a b) -> p w a b", a=WS, b=WS)
        usl = ut[r0:r0 + 28, :].rearrange("p (w a b) -> p w a b", a=WS, b=WS)
        relw_tmp = hpool.tile([14, TPC], BF16, tag="relw", name="relw_tmp", bufs=1)
        wsl = relw_tmp.rearrange("p (w a b) -> p w a b", a=WS, b=WS)
        qrow = (h % 2) * 64
        for side in range(2):
            for g in range(WS):
                src = qsl[:, :, g, :] if side == 0 else qsl[:, :, :, g]
                ps = ps_tile([WS, WPC * WS], "st")
                nc.tensor.matmul(
                    ps[:, :],
                    rhw_s[qrow:qrow + HD, g * 28 + side * 14:g * 28 + side * 14 + 14],
                    src, start=True, stop=True)
                if side == 0:
                    nc.scalar.copy(out=usl[0:14, :, g, :], in_=ps[:, :])
                else:
                    # dst base partition r0+14 is illegal for ACT/DVE: stage in
                    # a base-0 temp, then DMA (partition-unconstrained) to UT
                    nc.scalar.copy(out=wsl[:, :, :, g], in_=ps[:, :])
        nc.sync.dma_start(out=ut[r0 + 14:r0 + 28, :], in_=relw_tmp[:, :])

    # ================ P3-5: per window ================
    for w in range(WPC):
        c0 = w * N
        kt = [wpool.tile([128, N], BF16, tag=f"kt{j}", name=f"kt{j}") for j in range(NC6)]
        for j in range(NC6):
            ps = ps_tile([128, 512], "mm")
            for c in range(NC6):
                nc.tensor.matmul(ps[:, :N], wk_s[c][:, j * 128:(j + 1) * 128],
                                 xnT[c][:, c0:c0 + N],
                                 start=(c == 0), stop=(c == NC6 - 1))
            nc.scalar.activation(out=kt[j][:, :], in_=ps[:, :N],
                                 func=AF.Identity, bias=bias_s["bk"][:, j:j + 1])

        va = [wpool.tile([128, NH * 65], BF16, tag=f"va{m}", name=f"va{m}") for m in range(2)]
        for m, (m0, mrows) in enumerate(MROWS):
            for f in range(2):
                ps = ps_tile([128, 512], "mm")
                for c in range(NC6):
                    nc.tensor.matmul(ps[:mrows, :384],
                                     xnT[c][:, c0 + m0:c0 + m0 + mrows],
                                     wv_s[c][:, f * 384:(f + 1) * 384],
                                     start=(c == 0), stop=(c == NC6 - 1))
                dst = va[m].rearrange("p (h e) -> p h e", e=65)[
                    :mrows, f * 6:(f + 1) * 6, 0:64]
                nc.vector.tensor_add(
                    out=dst, in0=ps[:mrows, :384],
                    in1=bias_s["bv"][:mrows, f * 384:(f + 1) * 384])
            ones_cols = va[m].rearrange("p (h e) -> p h e", e=65)[:mrows, :, 64:65]
            nc.gpsimd.memset(ones_cols, 1.0)

        outT = [wpool.tile([128, N], BF16, tag=f"oT{j}", name=f"oT{j}") for j in range(NC6)]
        for h in range(NH):
            qtile, ktile = qT[h // 2], kt[h // 2]
            qrow = (h % 2) * 64
            ut, r0 = UT[h // 3], (h % 3) * 32
            pchunks = []
            for ci, (m0, mrows) in enumerate(MROWS):
                ps = ps_tile([128, N], "st" if ci == 0 else "tp")
                nc.tensor.matmul(ps[:mrows, :],
                                 ktile[qrow:qrow + 64, m0:m0 + mrows],
                                 qtile[qrow:qrow + 64, c0:c0 + N],
                                 start=True, stop=False, skip_group_check=True)
                nc.tensor.matmul(ps[:mrows, :], vmat_s[r0:r0 + 28, m0:m0 + mrows],
                                 ut[r0:r0 + 28, c0:c0 + N],
                                 start=False, stop=True, skip_group_check=True)
                pt = hpool.tile([128, N], BF16, tag="pexp")
                nc.scalar.activation(out=pt[:mrows], in_=ps[:mrows], func=AF.Exp)
                pchunks.append((pt, mrows))
            av = ps_tile([65, N], "av")
            for i, (pt, mrows) in enumerate(pchunks):
                nc.tensor.matmul(av[:, :], va[i][:mrows, h * 65:h * 65 + 65],
                                 pt[:mrows, :], start=(i == 0), stop=(i == 1),
                                 skip_group_check=True)
            rs = smal.tile([1, N], F32, tag="rs")
            nc.vector.reciprocal(out=rs, in_=av[64:65, :])
            rsb = smal.tile([64, N], F32, tag="rsb")
            nc.gpsimd.partition_broadcast(rsb, rs)
            nc.vector.tensor_mul(
                out=outT[h // 2][(h % 2) * 64:(h % 2) * 64 + 64, :],
                in0=av[0:64, :], in1=rsb)

        for m0, mrows in MROWS:
            tok_sb = hpool.tile([128, DIM], F32, tag="toksb")
            xres = hpool.tile([128, DIM], F32, tag="xres")
            nc.sync.dma_start(out=xres[:mrows], in_=x_d[c0 + m0:c0 + m0 + mrows, :])
            nc.vector.tensor_add(out=xres[:mrows], in0=xres[:mrows],
                                 in1=bias_s["bp"][:mrows, :])
            for f in range(2):
                ps = ps_tile([128, 512], "mm")
                for c in range(NC6):
                    nc.tensor.matmul(ps[:mrows, :384],
                                     outT[c][:, m0:m0 + mrows],
                                     wp_s[c][:, f * 384:(f + 1) * 384],
                                     start=(c == 0), stop=(c == NC6 - 1))
                nc.vector.tensor_add(out=tok_sb[:mrows, f * 384:(f + 1) * 384],
                                     in0=ps[:mrows, :384],
                                     in1=xres[:mrows, f * 384:(f + 1) * 384])
            nc.sync.dma_start(out=tok_d[c0 + m0:c0 + m0 + mrows, :],
                              in_=tok_sb[:mrows])

    # ================ P6: LN2 + MLP ================
    w1_s = load_w(d["w1_d"], NC6, 4 * DIM, lambda i: f"xnT{i}")
    w2_s = load_w(d["w2_d"], 24, DIM, lambda i: f"wg{i}")
    xn2T = [persist.tile([128, TPC], BF16, tag=f"qT{i}", name=f"xn2T{i}") for i in range(NC6)]

    ln_transpose_phase(tok_d, xn2T)

    for n0, ncols in NCHUNK:
        ht = [htpool.tile([128, 512], BF16, tag=f"ht{f}", name=f"ht{f}") for f in range(24)]
        for f in range(24):
            ps = ps_tile([128, 512], "mm")
            for c in range(NC6):
                nc.tensor.matmul(ps[:, :ncols], w1_s[c][:, f * 128:(f + 1) * 128],
                                 xn2T[c][:, n0:n0 + ncols],
                                 start=(c == 0), stop=(c == NC6 - 1))
            nc.scalar.activation(out=ht[f][:, :ncols], in_=ps[:, :ncols],
                                 func=AF.Gelu, bias=bias_s["b1"][:, f:f + 1])
        nsub = [(s * 128, min(128, ncols - s * 128))
                for s in range((ncols + 127) // 128)]
        for s0, srows in nsub:
            tok_sb = hpool.tile([128, DIM], F32, tag="toksb")
            nc.sync.dma_start(out=tok_sb[:srows],
                              in_=tok_d[n0 + s0:n0 + s0 + srows, :])
            nc.vector.tensor_add(out=tok_sb[:srows], in0=tok_sb[:srows],
                                 in1=bias_s["b2"][:srows, :])
            y_sb = hpool.tile([128, DIM], F32, tag="ysb")
            for half in range(2):
                ps = ps_tile([128, 512], "av")
                for f in range(24):
                    nc.tensor.matmul(ps[:srows, :384],
                                     ht[f][:, s0:s0 + srows],
                                     w2_s[f][:, half * 384:(half + 1) * 384],
                                     start=(f == 0), stop=(f == 23))
                nc.vector.tensor_add(
                    out=y_sb[:srows, half * 384:(half + 1) * 384],
                    in0=ps[:srows, :384],
                    in1=tok_sb[:srows, half * 384:(half + 1) * 384])
            nc.sync.dma_start(out=out_d[n0 + s0:n0 + s0 + srows, :],
                              in_=y_sb[:srows])
    ctx.close()


# ================= host side =================

_nc = None


def _get_nc():
    global _nc
    if _nc is None:
        _nc = build_program()
    return _nc


def _prep_host(inputs):
    f32 = np.float32
    qkv_w = np.asarray(inputs["qkv_w"], f32)
    qkv_b = np.asarray(inputs["qkv_b"], f32)
    ln1_w = np.asarray(inputs["ln1_w"], f32)
    ln1_b = np.asarray(inputs["ln1_b"], f32)
    ln2_w = np.asarray(inputs["ln2_w"], f32)
    ln2_b = np.asarray(inputs["ln2_b"], f32)
    Wq = (ln1_w[:, None] * qkv_w[:, :DIM]) / 8.0
    Wk = ln1_w[:, None] * qkv_w[:, DIM:2 * DIM]
    Wv = ln1_w[:, None] * qkv_w[:, 2 * DIM:]
    bq = (qkv_b[:DIM] + ln1_b @ qkv_w[:, :DIM]) / 8.0
    bk = qkv_b[DIM:2 * DIM] + ln1_b @ qkv_w[:, DIM:2 * DIM]
    bv = qkv_b[2 * DIM:] + ln1_b @ qkv_w[:, 2 * DIM:]
    fc1_w = np.asarray(inputs["fc1_w"], f32)
    W1 = ln2_w[:, None] * fc1_w
    b1 = np.asarray(inputs["fc1_b"], f32) + ln2_b @ fc1_w

    idx = np.arange(WS)[:, None] - np.arange(WS)[None, :] + (WS - 1)
    Rh = np.asarray(inputs["rel_pos_h"], f32)[idx]
    Rw = np.asarray(inputs["rel_pos_w"], f32)[idx]
    rhw = np.zeros((HD, WS, 2 * WS), f32)          # [c, g, side*14+k]
    rhw[:, :, :WS] = 8.0 * Rh.transpose(2, 0, 1)
    rhw[:, :, WS:] = 8.0 * Rw.transpose(2, 0, 1)
    rhw = rhw.reshape(HD, WS * 2 * WS)
    m = np.arange(N)
    vmat = np.zeros((2 * WS, N), f32)
    for j in range(WS):
        vmat[j, (m // WS) == j] = 1.0
        vmat[WS + j, (m % WS) == j] = 1.0

    bf = bfloat16
    return {
        "wq": Wq.astype(bf), "wk": Wk.astype(bf), "wv": Wv.astype(bf),
        "wp": np.asarray(inputs["proj_w"], f32).astype(bf),
        "w1": W1.astype(bf),
        "w2": np.asarray(inputs["fc2_w"], f32).astype(bf),
        "rhw": rhw.astype(bf), "vmat": vmat.astype(bf),
        "bq": np.ascontiguousarray(bq.reshape(NC6, 128).T),
        "bk": np.ascontiguousarray(bk.reshape(NC6, 128).T),
        "b1": np.ascontiguousarray(b1.reshape(4 * NC6, 128).T),
        "bv": bv.reshape(1, DIM).astype(bf),
        "bp": np.asarray(inputs["proj_b"], f32).reshape(1, DIM).astype(bf),
        "b2": np.asarray(inputs["fc2_b"], f32).reshape(1, DIM).astype(bf),
    }


def _window_partition(x):
    xp = np.zeros((B, 70, 70, DIM), np.float32)
    xp[:, :H, :W, :] = x
    xw = xp.reshape(B, NS, WS, NS, WS, DIM).transpose(0, 1, 3, 2, 4, 5)
    xw = xw.reshape(NWIN, N, DIM)
    full = np.zeros((NCORES * WPC, N, DIM), np.float32)
    full[:NWIN] = xw
    return full


def _window_unpartition(y):
    yw = y[:NWIN].reshape(B, NS, NS, WS, WS, DIM).transpose(0, 1, 3, 2, 4, 5)
    return np.ascontiguousarray(yw.reshape(B, 70, 70, DIM)[:, :H, :W, :])


def make_in_maps(inputs):
    x = np.asarray(inputs["x"], np.float32)
    xw = _window_partition(x)
    wts = _prep_host(inputs)
    in_maps = []
    for c in range(NCORES):
        m = dict(wts)
        m["xc"] = np.ascontiguousarray(
            xw[c * WPC:(c + 1) * WPC].reshape(TPC, DIM))
        in_maps.append(m)
    return in_maps


def run_on_hw(inputs, trace=False, **kw):
    nc = _get_nc()
    in_maps = make_in_maps(inputs)
    res = run_bass_kernel_spmd(nc, in_maps, core_ids=list(range(NCORES)),
                               trace=trace, **kw)
    out = np.zeros((NCORES * WPC, N, DIM), np.float32)
    for c in range(NCORES):
        out[c * WPC:(c + 1) * WPC] = res.results[c]["out"].reshape(WPC, N, DIM)
    return _window_unpartition(out), res


def kernel(x, ln1_w, ln1_b, qkv_w, qkv_b, proj_w, proj_b,
           rel_pos_h, rel_pos_w, ln2_w, ln2_b, fc1_w, fc1_b, fc2_w, fc2_b):
    inputs = dict(x=x, ln1_w=ln1_w, ln1_b=ln1_b, qkv_w=qkv_w, qkv_b=qkv_b,
                  proj_w=proj_w, proj_b=proj_b, rel_pos_h=rel_pos_h,
                  rel_pos_w=rel_pos_w, ln2_w=ln2_w, ln2_b=ln2_b,
                  fc1_w=fc1_w, fc1_b=fc1_b, fc2_w=fc2_w, fc2_b=fc2_b)
    out, _ = run_on_hw(inputs, trace=False)
    return out
